# revision 24
# baseline (speedup 1.0000x reference)
"""Sigmoid self-attention Bass kernel for Trainium2, SPMD on 8 cores. v8.

Problem: B=4, S=1024, F=256, H=8
  q = (X @ Wq).reshape(b,s,f,h); k,v likewise (self-attention)
  attn = sigmoid(sqrt(F) * q.kT) per (b,h);  wv = attn @ v
  out = relu(wv_flat @ Wo)

Sharding: data-parallel over (batch, seq-half): core c handles batch c//2,
query rows [half*512, half*512+512). V computed per-core for the full
sequence. No collectives.

Math: scores = q k^T = x (Wq Wk^T) x^T, with M_h = Wq_h Wk_h^T folded
on host. This removes the K projection and kT materialization entirely:
per head only qm = M_h^T x_q^T is computed ([256, 512]), and the scores
matmul contracts x^T (already resident for V) against qm.

attn = 0.5 + 0.5*tanh(8*qkt), so wv = 0.5*(colsum(V) + tanh(8*qkt) @ V).
The tanh term uses fp8(e4m3) DoubleRow matmuls (t8 @ v8). colsum(V) =
(sum_j x_j) @ Wv is precomputed on host per batch and added via the
per-partition-scalar port of DVE/ACT. The 0.5 is folded into Wo on host.
Output projection accumulates over heads in PSUM (memset once +
start=False throughout; PSUM zero-regions are bank-wide, so two groups
in one bank must never be mid-flight together).

Schedule: head h's block also emits qm/V for head h+1 so the PE never
waits on the DVE qt copy or the tanh chain. Per-DMA latency is ~2.5us
fixed, and HBM bandwidth is shared across queues, so the prologue keeps
all early transfers on one queue in consumption order (cross-queue
fair-sharing measurably delays the pipeline). Head 0's qm runs in
q-halves so compute starts on the first quarter of xq. Tail (head 7):
csum adds split in half, then per-q-chunk oproj -> relu
(Scalar/Vector) -> DMA (3 queues).
"""

import numpy as np

B, S, F, H = 4, 1024, 256, 8
N_CORES = 8
SQ = S // 2  # query rows per core

_CACHE = {}


def _build_nc():
    import concourse.mybir as mybir
    import concourse.tile as tile
    from concourse import bacc
    from concourse.tile_rust import add_dep_helper

    f32 = mybir.dt.float32
    rdt = mybir.dt.float32r
    f8 = mybir.dt.float8e4
    DR = mybir.MatmulPerfMode.DoubleRow
    Tanh = mybir.ActivationFunctionType.Tanh
    Relu = mybir.ActivationFunctionType.Relu
    Ident = mybir.ActivationFunctionType.Identity

    nc = bacc.Bacc()
    xqT = nc.declare_dram_parameter("xqT", [128, 2, SQ], rdt, isOutput=False)
    xoT = nc.declare_dram_parameter("xoT", [128, 2, SQ], rdt, isOutput=False)
    Wm = nc.declare_dram_parameter("Wm", [H, 128, 2, F], rdt, isOutput=False)
    Wv = nc.declare_dram_parameter("Wv", [H, 128, 2, F], rdt, isOutput=False)
    Wo = nc.declare_dram_parameter("Wo", [H, 128, 2, F], rdt, isOutput=False)
    csum_d = nc.declare_dram_parameter("csum", [128, 2 * H], f32, isOutput=False)
    out_d = nc.declare_dram_parameter("out", [SQ, F], f32, isOutput=True)

    with tile.TileContext(nc) as tc:
        with (
            tc.tile_pool(name="const", bufs=1) as const,
            tc.tile_pool(name="sb", bufs=2) as sb,
            tc.tile_pool(name="osb", bufs=1) as osb,
            tc.tile_pool(name="psB", bufs=3, space="PSUM") as psB,
            tc.tile_pool(name="psP", bufs=2, space="PSUM") as psP,
        ):
            # persistent activations: features on partitions, [128, kk, s]
            xq = const.tile([128, 2, SQ], rdt, name="xq", tag="xq")
            xo = const.tile([128, 2, SQ], rdt, name="xo", tag="xo")
            csum = const.tile([128, 2 * H], f32, name="csum", tag="csum")

            # persistent output-projection accumulators (PSUM, 2 banks):
            # PO[t][:, (mq%2)*256:...] accumulates q-chunk mq over all heads.
            # PSUM zero-regions are bank-wide: memset once, start=False
            # everywhere.
            PO = []
            po_msets = []
            for t in range(2):
                po = psP.tile([128, 512], f32, name=f"PO{t}", tag="po")
                po_msets.append(nc.vector.memset(po[:], 0.0))
                PO.append(po)

            state = {"prev_w_dma": None}

            def alloc_weights(h):
                return {
                    nm: sb.tile([128, 2, F], rdt, name=f"{nm}{h}", tag=nm, bufs=3)
                    for nm in ("wm", "wv", "wo")
                }

            def dma_weights(h):
                # weight tiles for head h; issued two heads ahead. Chain
                # transfers behind the previous head's last weight DMA so HBM
                # bandwidth isn't fair-shared across queues.
                ws = alloc_weights(h)
                dmas = []
                for nm, dram, eng in (
                    ("wm", Wm, nc.sync),
                    ("wv", Wv, nc.sync),
                    ("wo", Wo, nc.gpsimd),
                ):
                    d = eng.dma_start(out=ws[nm][:, :, :], in_=dram[h])
                    dmas.append(d)
                gate = state["prev_w_dma"].ins
                for d in dmas:
                    add_dep_helper(d.ins, gate, reason="hbm priority")
                state["prev_w_dma"] = dmas[-1]
                return ws

            def prologue_dmas(ws0, ws1):
                # HBM bandwidth is shared across queues: keep all early
                # transfers on ONE queue in consumption order (per-queue
                # transfers are serial and issue pipelines with transfer),
                # so the first matmul's deps arrive first. Per-DMA latency
                # is ~2.5us fixed + transfer, so the first x chunk is
                # quarter-sized to start compute earlier.
                nc.sync.dma_start(out=ws0["wm"][:, 0, :], in_=Wm[0][:, 0, :])
                nc.sync.dma_start(out=xq[:, 0, 0:256], in_=xqT[:, 0, 0:256])
                nc.sync.dma_start(out=ws0["wm"][:, 1, :], in_=Wm[0][:, 1, :])
                nc.sync.dma_start(out=xq[:, 1, 0:256], in_=xqT[:, 1, 0:256])
                nc.sync.dma_start(out=xq[:, 0, 256:512], in_=xqT[:, 0, 256:512])
                nc.sync.dma_start(out=xq[:, 1, 256:512], in_=xqT[:, 1, 256:512])
                d_wv0 = nc.sync.dma_start(out=ws0["wv"][:, :, :], in_=Wv[0])
                d_xo = []
                for kk in range(2):
                    d_xo.append(
                        nc.sync.dma_start(out=xo[:, kk, :], in_=xoT[:, kk, :])
                    )
                # scalar queue, gated behind xo1: wm1, wv1.
                d_wm1 = nc.scalar.dma_start(out=ws1["wm"][:, :, :], in_=Wm[1])
                d_wv1 = nc.scalar.dma_start(out=ws1["wv"][:, :, :], in_=Wv[1])
                for d in (d_wm1, d_wv1):
                    add_dep_helper(d.ins, d_xo[1].ins, reason="hbm priority")
                g = []
                g.append(nc.gpsimd.dma_start(out=csum[:], in_=csum_d[:]))
                g.append(nc.gpsimd.dma_start(out=ws0["wo"][:, :, :], in_=Wo[0]))
                g.append(nc.gpsimd.dma_start(out=ws1["wo"][:, :, :], in_=Wo[1]))
                for d in g:
                    add_dep_helper(d.ins, d_xo[1].ins, reason="hbm priority")
                state["prev_w_dma"] = g[-1]  # wo1
                # preload the tanh activation table while DMAs are in flight
                atl0 = const.tile([128, 1], f32, name="atl0", tag="atl0")
                atl1 = const.tile([128, 1], f32, name="atl1", tag="atl1")
                nc.vector.memset(atl0[:], 0.0)
                nc.scalar.activation(atl1[:], atl0[:], Tanh, scale=1.0)

            def qm_phase(h, ws, qsplit=1):
                # qm = M_h^T x_q^T : psq [128 (jout m), 2x512] -> qt f32r
                # qsplit>1 (head 0 only): smaller moving chunks so the first
                # matmul starts as soon as the first quarter of xq lands.
                # PSUM zero-regions are bank-wide: a start=True while another
                # group in the same bank is mid-accumulation wipes it. So each
                # q-chunk's full contraction completes before the next starts.
                psq = psB.tile([128, 1024], f32, name=f"psq{h}", tag="B")
                w = SQ // qsplit
                for c in range(qsplit):
                    sl = slice(c * w, (c + 1) * w)
                    for m in range(2):
                        for kk in range(2):
                            nc.tensor.matmul(
                                psq[:, m * 512 : (m + 1) * 512][:, sl],
                                ws["wm"][:, kk, m * 128 : (m + 1) * 128],
                                xq[:, kk, sl],
                                start=(kk == 0),
                                stop=(kk == 1),
                            )
                qt = sb.tile([128, 2, SQ], rdt, name=f"qt{h}", tag="qt")
                nc.vector.tensor_copy(qt[:, :, :], psq[:])
                return qt

            def v_phase(h, ws):
                # V for the full sequence: v8p[p] [128 j, 2, 256] fp8
                v8p = []
                for half in range(2):
                    VP = psB.tile([128, 1024], f32, name=f"VP{half}_{h}", tag="B")
                    for p2 in range(2):
                        p = half * 2 + p2
                        for t2 in range(2):
                            jb = p * 2 + t2
                            xh = xq if jb < 4 else xo
                            jj = jb % 4
                            for kk in range(2):
                                nc.tensor.matmul(
                                    VP[:, (p2 * 2 + t2) * 256 : (p2 * 2 + t2 + 1) * 256],
                                    xh[:, kk, jj * 128 : (jj + 1) * 128],
                                    ws["wv"][:, kk, :],
                                    start=(kk == 0),
                                    stop=(kk == 1),
                                )
                        v8 = sb.tile([128, 2, 256], f8, name=f"v8p{p}_{h}", tag=f"v8p{p}")
                        nc.vector.tensor_copy(
                            v8[:, :, :], VP[:, p2 * 512 : (p2 + 1) * 512]
                        )
                        v8p.append(v8)
                return v8p

            def scores(h, qt, p):
                # scores pair p -> psum [128 j(t2), 2, 512 q] -> tanh -> t8 fp8
                SP = psB.tile([128, 2, 512], f32, name=f"SP{p}_{h}", tag="B")
                for t2 in range(2):
                    jb = p * 2 + t2
                    xh = xq if jb < 4 else xo
                    jj = jb % 4
                    for m in range(2):
                        nc.tensor.matmul(
                            SP[:, t2, :],
                            xh[:, m, jj * 128 : (jj + 1) * 128],
                            qt[:, m, :],
                            start=(m == 0),
                            stop=(m == 1),
                        )
                t8 = sb.tile([128, 2, 512], f8, name=f"t8_{p}_{h}", tag=f"t8_{p}")
                nc.scalar.activation(t8[:, :, :], SP[:, :, :], Tanh, scale=8.0)
                return t8

            def ot_contrib(v8p, OMB, p, t8s, first=None, stop=None):
                # each m-half of OMB is its own accumulation group
                if first is None:
                    first = p == 0
                if stop is None:
                    stop = p == 3
                for m in range(2):
                    nc.tensor.matmul(
                        OMB[:, m * 512 : (m + 1) * 512],
                        v8p[p][:, :, m * 128 : (m + 1) * 128],
                        t8s[p][:, :, :],
                        start=first,
                        stop=stop,
                        perf_mode=DR,
                    )

            def csum_add(h, OMB, dest, split):
                # ot = OMB + colsum (per-partition scalar), f32r.
                # split: column chunks per m-half (subtile unlock of oproj).
                w = 512 // split
                for c in range(split):
                    sl = slice(c * w, (c + 1) * w)
                    nc.vector.tensor_scalar_add(
                        dest[0][:, sl], OMB[:, 0:512][:, sl],
                        csum[:, 2 * h : 2 * h + 1],
                    )
                    nc.scalar.activation(
                        dest[1][:, sl], OMB[:, 512:1024][:, sl], Ident,
                        bias=csum[:, 2 * h + 1 : 2 * h + 2],
                    )

            def oproj(h, ot, wo):
                for mq in range(4):
                    po = PO[mq // 2][:, (mq % 2) * 256 : (mq % 2 + 1) * 256]
                    for m in range(2):
                        mm = nc.tensor.matmul(
                            po,
                            ot[m][:, mq * 128 : (mq + 1) * 128],
                            wo[:, m, :],
                            start=False,
                            stop=False,
                            skip_group_check=True,
                        )
                        if h == 0:
                            add_dep_helper(
                                mm.ins, po_msets[mq // 2].ins,
                                reason="po zeroed before accumulation",
                            )

            def tail_epilogue(ot7, wo):
                # oproj(h7, stop) -> relu -> DMA, pipelined per q-chunk
                dma_eng = [nc.sync, nc.gpsimd, nc.scalar, nc.sync]
                for mq in range(4):
                    po = PO[mq // 2][:, (mq % 2) * 256 : (mq % 2 + 1) * 256]
                    for m in range(2):
                        nc.tensor.matmul(
                            po,
                            ot7[m][:, mq * 128 : (mq + 1) * 128],
                            wo[:, m, :],
                            start=False,
                            stop=(m == 1),
                            skip_group_check=True,
                        )
                    o = osb.tile([128, F], f32, name=f"outsb{mq}", tag=f"outsb{mq}")
                    if mq % 2 == 0:
                        nc.scalar.activation(o[:], po, Relu)
                    else:
                        nc.vector.tensor_scalar_max(o[:], po, 0.0)
                    dma_eng[mq].dma_start(
                        out=out_d[mq * 128 : (mq + 1) * 128, :], in_=o[:]
                    )

            # ---------------- main pipeline ----------------
            ws = [alloc_weights(0), alloc_weights(1)] + [None] * (H - 2)
            prologue_dmas(ws[0], ws[1])
            qt_cur = qm_phase(0, ws[0], qsplit=2)
            v8_cur = v_phase(0, ws[0])

            for h in range(H):
                last = h == H - 1
                if h + 2 < H:
                    ws[h + 2] = dma_weights(h + 2)

                t8s = [scores(h, qt_cur, p) for p in range(4)]
                OMB = psB.tile([128, 1024], f32, name=f"OMB{h}", tag="B")
                ot_contrib(v8_cur, OMB, 0, t8s)
                ot_contrib(v8_cur, OMB, 1, t8s)
                if not last:
                    qt_nxt = qm_phase(h + 1, ws[h + 1])
                ot_contrib(v8_cur, OMB, 2, t8s)
                ot_contrib(v8_cur, OMB, 3, t8s)

                ot = [
                    sb.tile([128, SQ], rdt, name=f"ot{m}_{h}", tag=f"ot{m}")
                    for m in range(2)
                ]
                csum_add(h, OMB, ot, split=(2 if last else 1))

                if not last:
                    v8_nxt = v_phase(h + 1, ws[h + 1])
                    oproj(h, ot, ws[h]["wo"])
                    qt_cur, v8_cur = qt_nxt, v8_nxt
                else:
                    tail_epilogue(ot, ws[h]["wo"])

    nc.finalize()
    return nc


def _get_nc():
    if "nc" not in _CACHE:
        _CACHE["nc"] = _build_nc()
    return _CACHE["nc"]


def _prep_weights(Wq, Wk, Wv, Wo):
    # [F, F*H] with column f_out*H+h  ->  per-head [f_in, f_out]
    wqh = Wq.reshape(F, F, H).transpose(2, 0, 1)  # [H, f_in, f_out]
    wkh = Wk.reshape(F, F, H).transpose(2, 0, 1)
    wvh = np.ascontiguousarray(Wv.reshape(F, F, H).transpose(2, 0, 1))
    # M_h = Wq_h @ Wk_h^T : [H, f_in_q, f_in_k]
    M = np.matmul(np.ascontiguousarray(wqh), wkh.transpose(0, 2, 1))
    # [H, 256, 256] -> [H, 128, 2, 256] (partition-major chunk interleave)
    def lay(w):
        return np.ascontiguousarray(
            w.reshape(H, 2, 128, F).transpose(0, 2, 1, 3)
        )
    # [F*H, F] with row f*H+h -> [H, F, F]; fold the 0.5 centering factor
    woh = Wo.reshape(F, H, F).transpose(1, 0, 2) * 0.5
    return lay(M), lay(wvh), lay(woh), wvh


def kernel(q_input, Wq, Wk, Wv, Wo, _trace=False):
    from concourse.bass_utils import run_bass_kernel_spmd

    nc = _get_nc()
    wm, wv, wo, wvh = _prep_weights(
        np.asarray(Wq, np.float32),
        np.asarray(Wk, np.float32),
        np.asarray(Wv, np.float32),
        np.asarray(Wo, np.float32),
    )
    q_input = np.asarray(q_input, np.float32)

    in_maps = []
    for c in range(N_CORES):
        b, half = c // 2, c % 2
        xT = q_input[b].T  # [F, S]

        def lay_x(cols):
            return np.ascontiguousarray(
                cols.reshape(2, 128, SQ).transpose(1, 0, 2)
            )
        xqT = lay_x(xT[:, half * SQ : (half + 1) * SQ])
        xoT = lay_x(xT[:, (1 - half) * SQ : (2 - half) * SQ])
        # colsum_h = (sum_j x[j,:]) @ Wv_h ; layout [128, h*2+m]
        xsum = q_input[b].sum(axis=0)  # [F]
        cs = (xsum @ wvh).reshape(H, 2, 128)
        csum = np.ascontiguousarray(cs.transpose(2, 0, 1).reshape(128, 2 * H))
        in_maps.append(
            {
                "xqT": xqT,
                "xoT": xoT,
                "Wm": wm,
                "Wv": wv,
                "Wo": wo,
                "csum": csum,
            }
        )

    res = run_bass_kernel_spmd(nc, in_maps, list(range(N_CORES)), trace=_trace)

    out = np.empty((B, S, F), np.float32)
    for c in range(N_CORES):
        b, half = c // 2, c % 2
        out[b, half * SQ : (half + 1) * SQ, :] = res.results[c]["out"]
    if _trace:
        return out, res
    return out


# revision 25
# speedup vs baseline: 2.6340x; 2.6340x over previous
"""Sigmoid self-attention Bass kernel for Trainium2, SPMD on 8 cores. v8.

Problem: B=4, S=1024, F=256, H=8
  q = (X @ Wq).reshape(b,s,f,h); k,v likewise (self-attention)
  attn = sigmoid(sqrt(F) * q.kT) per (b,h);  wv = attn @ v
  out = relu(wv_flat @ Wo)

Sharding: data-parallel over (batch, seq-half): core c handles batch c//2,
query rows [half*512, half*512+512). V computed per-core for the full
sequence. No collectives.

Math: scores = q k^T = x (Wq Wk^T) x^T, with M_h = Wq_h Wk_h^T folded
on host. This removes the K projection and kT materialization entirely:
per head only qm = M_h^T x_q^T is computed ([256, 512]), and the scores
matmul contracts x^T (already resident for V) against qm.

attn = 0.5 + 0.5*tanh(8*qkt), so wv = 0.5*(colsum(V) + tanh(8*qkt) @ V).
The tanh term uses fp8(e4m3) DoubleRow matmuls (t8 @ v8). colsum(V) =
(sum_j x_j) @ Wv is precomputed on host per batch and added via the
per-partition-scalar port of DVE/ACT. The 0.5 is folded into Wo on host.
Output projection accumulates over heads in PSUM (memset once +
start=False throughout; PSUM zero-regions are bank-wide, so two groups
in one bank must never be mid-flight together).

Schedule: head h's block also emits qm/V for head h+1 so the PE never
waits on the DVE qt copy or the tanh chain. Per-DMA latency is ~2.5us
fixed, and HBM bandwidth is shared across queues, so the prologue keeps
all early transfers on one queue in consumption order (cross-queue
fair-sharing measurably delays the pipeline). Head 0's qm runs in
q-halves so compute starts on the first quarter of xq. Tail (head 7):
csum adds split in half, then per-q-chunk oproj -> relu
(Scalar/Vector) -> DMA (3 queues).
"""

import numpy as np

B, S, F, H = 4, 1024, 256, 8
N_CORES = 8
SQ = S // 2  # query rows per core

_CACHE = {}


def _build_nc():
    import concourse.mybir as mybir
    import concourse.tile as tile
    from concourse import bacc
    from concourse.tile_rust import add_dep_helper

    f32 = mybir.dt.float32
    rdt = mybir.dt.float32r
    f8 = mybir.dt.float8e4
    DR = mybir.MatmulPerfMode.DoubleRow
    Tanh = mybir.ActivationFunctionType.Tanh
    Relu = mybir.ActivationFunctionType.Relu
    Ident = mybir.ActivationFunctionType.Identity

    nc = bacc.Bacc()
    xqT = nc.declare_dram_parameter("xqT", [128, 2, SQ], rdt, isOutput=False)
    xoT = nc.declare_dram_parameter("xoT", [128, 2, SQ], rdt, isOutput=False)
    Wm = nc.declare_dram_parameter("Wm", [H, 128, 2, F], rdt, isOutput=False)
    Wv = nc.declare_dram_parameter("Wv", [H, 128, 2, F], rdt, isOutput=False)
    Wo = nc.declare_dram_parameter("Wo", [H, 128, 2, F], rdt, isOutput=False)
    csum_d = nc.declare_dram_parameter("csum", [128, 2 * H], f32, isOutput=False)
    out_d = nc.declare_dram_parameter("out", [SQ, F], f32, isOutput=True)

    with tile.TileContext(nc) as tc:
        with (
            tc.tile_pool(name="const", bufs=1) as const,
            tc.tile_pool(name="sb", bufs=2) as sb,
            tc.tile_pool(name="osb", bufs=1) as osb,
            tc.tile_pool(name="psB", bufs=3, space="PSUM") as psB,
            tc.tile_pool(name="psP", bufs=2, space="PSUM") as psP,
        ):
            # persistent activations: features on partitions, [128, kk, s]
            xq = const.tile([128, 2, SQ], rdt, name="xq", tag="xq")
            xo = const.tile([128, 2, SQ], rdt, name="xo", tag="xo")
            csum = const.tile([128, 2 * H], f32, name="csum", tag="csum")

            # persistent output-projection accumulators (PSUM, 2 banks):
            # PO[t][:, (mq%2)*256:...] accumulates q-chunk mq over all heads.
            # PSUM zero-regions are bank-wide: memset once, start=False
            # everywhere.
            PO = []
            po_msets = []
            for t in range(2):
                po = psP.tile([128, 512], f32, name=f"PO{t}", tag="po")
                po_msets.append(nc.vector.memset(po[:], 0.0))
                PO.append(po)

            state = {"prev_w_dma": None}

            def alloc_weights(h):
                return {
                    nm: sb.tile([128, 2, F], rdt, name=f"{nm}{h}", tag=nm, bufs=3)
                    for nm in ("wm", "wv", "wo")
                }

            def dma_weights(h):
                # weight tiles for head h; issued two heads ahead. Chain
                # transfers behind the previous head's last weight DMA so HBM
                # bandwidth isn't fair-shared across queues.
                ws = alloc_weights(h)
                dmas = []
                for nm, dram, eng in (
                    ("wm", Wm, nc.sync),
                    ("wv", Wv, nc.sync),
                    ("wo", Wo, nc.gpsimd),
                ):
                    d = eng.dma_start(out=ws[nm][:, :, :], in_=dram[h])
                    dmas.append(d)
                gate = state["prev_w_dma"].ins
                for d in dmas:
                    add_dep_helper(d.ins, gate, reason="hbm priority")
                state["prev_w_dma"] = dmas[-1]
                return ws

            def prologue_dmas(ws0, ws1):
                # HBM bandwidth is shared across queues: keep all early
                # transfers on ONE queue in consumption order (per-queue
                # transfers are serial and issue pipelines with transfer),
                # so the first matmul's deps arrive first. Per-DMA latency
                # is ~2.5us fixed + transfer, so the first x chunk is
                # quarter-sized to start compute earlier.
                nc.sync.dma_start(out=ws0["wm"][:, 0, :], in_=Wm[0][:, 0, :])
                nc.sync.dma_start(out=xq[:, 0, 0:256], in_=xqT[:, 0, 0:256])
                nc.sync.dma_start(out=ws0["wm"][:, 1, :], in_=Wm[0][:, 1, :])
                nc.sync.dma_start(out=xq[:, 1, 0:256], in_=xqT[:, 1, 0:256])
                nc.sync.dma_start(out=xq[:, 0, 256:512], in_=xqT[:, 0, 256:512])
                nc.sync.dma_start(out=xq[:, 1, 256:512], in_=xqT[:, 1, 256:512])
                d_wv0 = nc.sync.dma_start(out=ws0["wv"][:, :, :], in_=Wv[0])
                d_xo = []
                for kk in range(2):
                    d_xo.append(
                        nc.sync.dma_start(out=xo[:, kk, :], in_=xoT[:, kk, :])
                    )
                # scalar queue, gated behind xo1: wm1, wv1.
                d_wm1 = nc.scalar.dma_start(out=ws1["wm"][:, :, :], in_=Wm[1])
                d_wv1 = nc.scalar.dma_start(out=ws1["wv"][:, :, :], in_=Wv[1])
                for d in (d_wm1, d_wv1):
                    add_dep_helper(d.ins, d_xo[1].ins, reason="hbm priority")
                g = []
                g.append(nc.gpsimd.dma_start(out=csum[:], in_=csum_d[:]))
                g.append(nc.gpsimd.dma_start(out=ws0["wo"][:, :, :], in_=Wo[0]))
                g.append(nc.gpsimd.dma_start(out=ws1["wo"][:, :, :], in_=Wo[1]))
                for d in g:
                    add_dep_helper(d.ins, d_xo[1].ins, reason="hbm priority")
                state["prev_w_dma"] = g[-1]  # wo1
                # preload the tanh activation table while DMAs are in flight
                atl0 = const.tile([128, 1], f32, name="atl0", tag="atl0")
                atl1 = const.tile([128, 1], f32, name="atl1", tag="atl1")
                nc.vector.memset(atl0[:], 0.0)
                nc.scalar.activation(atl1[:], atl0[:], Tanh, scale=1.0)

            def qm_phase(h, ws, qsplit=1):
                # qm = M_h^T x_q^T : psq [128 (jout m), 2x512] -> qt f32r
                # qsplit>1 (head 0 only): smaller moving chunks so the first
                # matmul starts as soon as the first quarter of xq lands.
                # PSUM zero-regions are bank-wide: a start=True while another
                # group in the same bank is mid-accumulation wipes it. So each
                # q-chunk's full contraction completes before the next starts.
                psq = psB.tile([128, 1024], f32, name=f"psq{h}", tag="B")
                w = SQ // qsplit
                for c in range(qsplit):
                    sl = slice(c * w, (c + 1) * w)
                    for m in range(2):
                        for kk in range(2):
                            nc.tensor.matmul(
                                psq[:, m * 512 : (m + 1) * 512][:, sl],
                                ws["wm"][:, kk, m * 128 : (m + 1) * 128],
                                xq[:, kk, sl],
                                start=(kk == 0),
                                stop=(kk == 1),
                            )
                qt = sb.tile([128, 2, SQ], rdt, name=f"qt{h}", tag="qt")
                nc.vector.tensor_copy(qt[:, :, :], psq[:])
                return qt

            def v_phase(h, ws):
                # V for the full sequence: v8p[p] [128 j, 2, 256] fp8
                v8p = []
                for half in range(2):
                    VP = psB.tile([128, 1024], f32, name=f"VP{half}_{h}", tag="B")
                    for p2 in range(2):
                        p = half * 2 + p2
                        for t2 in range(2):
                            jb = p * 2 + t2
                            xh = xq if jb < 4 else xo
                            jj = jb % 4
                            for kk in range(2):
                                nc.tensor.matmul(
                                    VP[:, (p2 * 2 + t2) * 256 : (p2 * 2 + t2 + 1) * 256],
                                    xh[:, kk, jj * 128 : (jj + 1) * 128],
                                    ws["wv"][:, kk, :],
                                    start=(kk == 0),
                                    stop=(kk == 1),
                                )
                        v8 = sb.tile([128, 2, 256], f8, name=f"v8p{p}_{h}", tag=f"v8p{p}")
                        nc.vector.tensor_copy(
                            v8[:, :, :], VP[:, p2 * 512 : (p2 + 1) * 512]
                        )
                        v8p.append(v8)
                return v8p

            def scores(h, qt, p):
                # scores pair p -> psum [128 j(t2), 2, 512 q] -> tanh -> t8 fp8
                SP = psB.tile([128, 2, 512], f32, name=f"SP{p}_{h}", tag="B")
                for t2 in range(2):
                    jb = p * 2 + t2
                    xh = xq if jb < 4 else xo
                    jj = jb % 4
                    for m in range(2):
                        nc.tensor.matmul(
                            SP[:, t2, :],
                            xh[:, m, jj * 128 : (jj + 1) * 128],
                            qt[:, m, :],
                            start=(m == 0),
                            stop=(m == 1),
                        )
                t8 = sb.tile([128, 2, 512], f8, name=f"t8_{p}_{h}", tag=f"t8_{p}")
                nc.scalar.activation(t8[:, :, :], SP[:, :, :], Tanh, scale=8.0)
                return t8

            def ot_contrib(v8p, OMB, p, t8s, first=None, stop=None):
                # each m-half of OMB is its own accumulation group
                if first is None:
                    first = p == 0
                if stop is None:
                    stop = p == 3
                for m in range(2):
                    nc.tensor.matmul(
                        OMB[:, m * 512 : (m + 1) * 512],
                        v8p[p][:, :, m * 128 : (m + 1) * 128],
                        t8s[p][:, :, :],
                        start=first,
                        stop=stop,
                        perf_mode=DR,
                    )

            def csum_add(h, OMB, dest, split):
                # ot = OMB + colsum (per-partition scalar), f32r.
                # split: column chunks per m-half (subtile unlock of oproj).
                w = 512 // split
                for c in range(split):
                    sl = slice(c * w, (c + 1) * w)
                    nc.vector.tensor_scalar_add(
                        dest[0][:, sl], OMB[:, 0:512][:, sl],
                        csum[:, 2 * h : 2 * h + 1],
                    )
                    nc.scalar.activation(
                        dest[1][:, sl], OMB[:, 512:1024][:, sl], Ident,
                        bias=csum[:, 2 * h + 1 : 2 * h + 2],
                    )

            def oproj(h, ot, wo):
                for mq in range(4):
                    po = PO[mq // 2][:, (mq % 2) * 256 : (mq % 2 + 1) * 256]
                    for m in range(2):
                        mm = nc.tensor.matmul(
                            po,
                            ot[m][:, mq * 128 : (mq + 1) * 128],
                            wo[:, m, :],
                            start=False,
                            stop=False,
                            skip_group_check=True,
                        )
                        if h == 0:
                            add_dep_helper(
                                mm.ins, po_msets[mq // 2].ins,
                                reason="po zeroed before accumulation",
                            )

            def tail_epilogue(ot7, wo):
                # oproj(h7, stop) -> relu -> DMA, pipelined per q-chunk
                dma_eng = [nc.sync, nc.gpsimd, nc.scalar, nc.sync]
                for mq in range(4):
                    po = PO[mq // 2][:, (mq % 2) * 256 : (mq % 2 + 1) * 256]
                    for m in range(2):
                        nc.tensor.matmul(
                            po,
                            ot7[m][:, mq * 128 : (mq + 1) * 128],
                            wo[:, m, :],
                            start=False,
                            stop=(m == 1),
                            skip_group_check=True,
                        )
                    o = osb.tile([128, F], f32, name=f"outsb{mq}", tag=f"outsb{mq}")
                    if mq % 2 == 0:
                        nc.scalar.activation(o[:], po, Relu)
                    else:
                        nc.vector.tensor_scalar_max(o[:], po, 0.0)
                    dma_eng[mq].dma_start(
                        out=out_d[mq * 128 : (mq + 1) * 128, :], in_=o[:]
                    )

            # ---------------- main pipeline ----------------
            ws = [alloc_weights(0), alloc_weights(1)] + [None] * (H - 2)
            prologue_dmas(ws[0], ws[1])
            qt_cur = qm_phase(0, ws[0], qsplit=2)
            v8_cur = v_phase(0, ws[0])

            for h in range(H):
                last = h == H - 1
                if h + 2 < H:
                    ws[h + 2] = dma_weights(h + 2)

                t8s = [scores(h, qt_cur, p) for p in range(4)]
                OMB = psB.tile([128, 1024], f32, name=f"OMB{h}", tag="B")
                ot_contrib(v8_cur, OMB, 0, t8s)
                ot_contrib(v8_cur, OMB, 1, t8s)
                if not last:
                    qt_nxt = qm_phase(h + 1, ws[h + 1])
                    ot_contrib(v8_cur, OMB, 2, t8s)
                    ot_contrib(v8_cur, OMB, 3, t8s)
                else:
                    # end on p=2: its tanh finished ~1us before p=3's, so the
                    # tail chain doesn't wait on the freshest activation
                    ot_contrib(v8_cur, OMB, 3, t8s, first=False, stop=False)
                    ot_contrib(v8_cur, OMB, 2, t8s, first=False, stop=True)

                ot = [
                    sb.tile([128, SQ], rdt, name=f"ot{m}_{h}", tag=f"ot{m}")
                    for m in range(2)
                ]
                csum_add(h, OMB, ot, split=(2 if last else 1))

                if not last:
                    v8_nxt = v_phase(h + 1, ws[h + 1])
                    oproj(h, ot, ws[h]["wo"])
                    qt_cur, v8_cur = qt_nxt, v8_nxt
                else:
                    tail_epilogue(ot, ws[h]["wo"])

    nc.finalize()
    return nc


def _get_nc():
    if "nc" not in _CACHE:
        _CACHE["nc"] = _build_nc()
    return _CACHE["nc"]


def _prep_weights(Wq, Wk, Wv, Wo):
    # [F, F*H] with column f_out*H+h  ->  per-head [f_in, f_out]
    wqh = Wq.reshape(F, F, H).transpose(2, 0, 1)  # [H, f_in, f_out]
    wkh = Wk.reshape(F, F, H).transpose(2, 0, 1)
    wvh = np.ascontiguousarray(Wv.reshape(F, F, H).transpose(2, 0, 1))
    # M_h = Wq_h @ Wk_h^T : [H, f_in_q, f_in_k]
    M = np.matmul(np.ascontiguousarray(wqh), wkh.transpose(0, 2, 1))
    # [H, 256, 256] -> [H, 128, 2, 256] (partition-major chunk interleave)
    def lay(w):
        return np.ascontiguousarray(
            w.reshape(H, 2, 128, F).transpose(0, 2, 1, 3)
        )
    # [F*H, F] with row f*H+h -> [H, F, F]; fold the 0.5 centering factor
    woh = Wo.reshape(F, H, F).transpose(1, 0, 2) * 0.5
    return lay(M), lay(wvh), lay(woh), wvh


def kernel(q_input, Wq, Wk, Wv, Wo, _trace=False):
    from concourse.bass_utils import run_bass_kernel_spmd

    nc = _get_nc()
    wm, wv, wo, wvh = _prep_weights(
        np.asarray(Wq, np.float32),
        np.asarray(Wk, np.float32),
        np.asarray(Wv, np.float32),
        np.asarray(Wo, np.float32),
    )
    q_input = np.asarray(q_input, np.float32)

    in_maps = []
    for c in range(N_CORES):
        b, half = c // 2, c % 2
        xT = q_input[b].T  # [F, S]

        def lay_x(cols):
            return np.ascontiguousarray(
                cols.reshape(2, 128, SQ).transpose(1, 0, 2)
            )
        xqT = lay_x(xT[:, half * SQ : (half + 1) * SQ])
        xoT = lay_x(xT[:, (1 - half) * SQ : (2 - half) * SQ])
        # colsum_h = (sum_j x[j,:]) @ Wv_h ; layout [128, h*2+m]
        xsum = q_input[b].sum(axis=0)  # [F]
        cs = (xsum @ wvh).reshape(H, 2, 128)
        csum = np.ascontiguousarray(cs.transpose(2, 0, 1).reshape(128, 2 * H))
        in_maps.append(
            {
                "xqT": xqT,
                "xoT": xoT,
                "Wm": wm,
                "Wv": wv,
                "Wo": wo,
                "csum": csum,
            }
        )

    res = run_bass_kernel_spmd(nc, in_maps, list(range(N_CORES)), trace=_trace)

    out = np.empty((B, S, F), np.float32)
    for c in range(N_CORES):
        b, half = c // 2, c % 2
        out[b, half * SQ : (half + 1) * SQ, :] = res.results[c]["out"]
    if _trace:
        return out, res
    return out
